# revision 25
# baseline (speedup 1.0000x reference)
"""IntraAttention Trainium2 kernel, 8-core SPMD, fp8 DoubleRow edition.

Reference computation (N=4096 rows, d=1024):
    Q = X @ Wq.T + bq ; K = X @ Wk.T + bk ; V = X @ Wv.T + bv
    alpha = softmax(Q @ K.T / sqrt(d), axis=1)
    V_ = alpha @ V
    x = concat([V_, Q], axis=1)              # [N, 2d]
    x1 = x @ Wl.T + bl                        # [N, d]
    h = x @ Wa.T + ba                         # [N, 2d]
    out = x1 * (h[:, :d] * sigmoid(h[:, d:]))

Sharding: rows of X sharded across 8 cores (512 rows each). Q local;
K and V shards all-gathered in fp8 (K in a swizzled [p, (kt, c, n)]
layout so per-key-tile loads are contiguous), one collective per
tensor. All error-tolerant matmuls (K/V projections, scores, alpha@V)
run fp8e4 DoubleRow (256-deep contraction per MM); the Q projection
and the concat->linear->GLU chain stay fp16. While the gathers fly,
the PE precomputes the full Q-half of x1 and both halves of h. All
weights are loaded before the gather completions so the collective
receive traffic and the K/V tile loads get the DMA engines to
themselves.
"""

import numpy as np
import ml_dtypes

import concourse.bass as bass
import concourse.bacc as bacc
import concourse.tile as tile
import concourse.bass_utils as bass_utils
from concourse import mybir

P = 128            # partitions
D = 1024           # model dim
N = 4096           # rows
NCORES = 8
R = N // NCORES    # rows per core = 512
DC = D // P        # d chunks = 8
TD = 2 * D         # 2048
TDC = TD // P      # 16
NPAIR = 16         # global key-tile pairs (256 keys each)

F32 = mybir.dt.float32
F16 = mybir.dt.float16
F8 = mybir.dt.float8e4
DR = mybir.MatmulPerfMode.DoubleRow

RG = [list(range(NCORES))]


def build_nc():
    nc = bacc.Bacc(
        "TRN2",
        target_bir_lowering=False,
        debug=False,
        num_devices=NCORES,
    )

    # ---- per-core I/O ----
    xt = nc.dram_tensor("xt", [D, R], F16, kind="ExternalInput")      # X_c.T
    xt8 = nc.dram_tensor("xt8", [D, R], F8, kind="ExternalInput")     # X_c.T
    wqt = nc.dram_tensor("wqt", [D, D], F16, kind="ExternalInput")    # Wq.T
    wk8 = nc.dram_tensor("wk8", [D, D], F8, kind="ExternalInput")     # Wk.T
    wv8 = nc.dram_tensor("wv8", [D, D], F8, kind="ExternalInput")     # Wv.T
    wlt = nc.dram_tensor("wlt", [TD, D], F16, kind="ExternalInput")   # Wl.T
    wat = nc.dram_tensor("wat", [TD, TD], F16, kind="ExternalInput")  # Wa.T
    bq = nc.dram_tensor("bq", [P, DC], F32, kind="ExternalInput")
    bk = nc.dram_tensor("bk", [P, DC], F32, kind="ExternalInput")
    bvb = nc.dram_tensor("bvb", [P, D], F32, kind="ExternalInput")    # bv bcast
    bl = nc.dram_tensor("bl", [P, DC], F32, kind="ExternalInput")
    ba = nc.dram_tensor("ba", [P, TDC], F32, kind="ExternalInput")
    out = nc.dram_tensor("out", [D, R], F32, kind="ExternalOutput")   # out_c.T

    # ---- collective buffers (fp8) ----
    # K.T swizzled: row p holds [kt, c, n] so a key-tile load is one
    # contiguous 1KB-per-partition stretch.
    kst_d = [nc.dram_tensor(f"kst_d{h}", [P, 2 * DC * P], F8)
             for h in range(2)]
    ag_k = [nc.dram_tensor(f"ag_k{h}", [NCORES * P, 2 * DC * P], F8,
                           addr_space="Shared") for h in range(2)]
    vc_d = nc.dram_tensor("vc_d", [R, D], F8)
    ag_v = nc.dram_tensor("ag_v", [NCORES * R, D], F8, addr_space="Shared")

    with tile.TileContext(nc) as tc:
        with (
            tc.tile_pool(name="cpool", bufs=1) as cpool,
            tc.tile_pool(name="pspool", bufs=8, space="PSUM") as pspool,
        ):
            # constants on the scalar DMA queue
            bq_t = cpool.tile([P, DC], F32, name="bq_t")
            bk_t = cpool.tile([P, DC], F32, name="bk_t")
            bl_t = cpool.tile([P, DC], F32, name="bl_t")
            ba_t = cpool.tile([P, TDC], F32, name="ba_t")
            bvb_t = cpool.tile([P, D], F32, name="bvb_t")
            nc.scalar.dma_start(bq_t, bq[:, :])
            nc.scalar.dma_start(bk_t, bk[:, :])
            nc.scalar.dma_start(bl_t, bl[:, :])
            nc.scalar.dma_start(ba_t, ba[:, :])
            nc.scalar.dma_start(bvb_t, bvb[:, :])
            ones2_t = cpool.tile([P, 2, 16], F8, name="ones2_t")
            nc.vector.memset(ones2_t, 1.0)
            ones_row = cpool.tile([1, P], F16, name="ones_row")
            nc.vector.memset(ones_row, 1.0)

            with tc.tile_pool(name="qpool", bufs=1) as qpool, \
                 tc.tile_pool(name="vtpool", bufs=1) as vtpool, \
                 tc.tile_pool(name="qfpool", bufs=1) as qfpool, \
                 tc.tile_pool(name="vwpool", bufs=1) as vwpool, \
                 tc.tile_pool(name="stpool", bufs=8) as stpool:

                # ============ K/V/Q projections + allgathers ============
                with tc.tile_pool(name="xpool", bufs=1) as xpool, \
                     tc.tile_pool(name="kvwpool", bufs=1) as kvwpool, \
                     tc.tile_pool(name="kstpool", bufs=1) as kstpool:

                    # --- K_c.T = Wk @ X_c.T + bk  (fp8 DoubleRow) ---
                    xt8_t = [xpool.tile([P, 2, R], F8, name=f"xt8_{j}")
                             for j in range(4)]
                    wk8_t = [kvwpool.tile([P, 2, D], F8, name=f"wk8_{j}")
                             for j in range(4)]
                    wv8_t = [kvwpool.tile([P, 2, D], F8, name=f"wv8_{j}")
                             for j in range(4)]

                    def ld_pair(dst, src, j):
                        nc.sync.dma_start(
                            dst,
                            src[2 * j * P:(2 * j + 2) * P, :]
                            .rearrange("(s p) c -> p s c", p=P))

                    ld_pair(xt8_t[0], xt8, 0)
                    ld_pair(wk8_t[0], wk8, 0)
                    for j in range(1, 4):
                        ld_pair(xt8_t[j], xt8, j)
                        ld_pair(wk8_t[j], wk8, j)
                    for j in range(4):
                        ld_pair(wv8_t[j], wv8, j)

                    # fp16 Q operands on the scalar queue (runs in parallel)
                    xt_t = [xpool.tile([P, R], F16, name=f"xt_{k}")
                            for k in range(DC)]
                    wq_t = [kvwpool.tile([P, D], F16, name=f"wq_{k}")
                            for k in range(DC)]
                    for k in range(DC):
                        nc.scalar.dma_start(xt_t[k], xt[k * P:(k + 1) * P, :])
                        nc.scalar.dma_start(wq_t[k], wqt[k * P:(k + 1) * P, :])

                    # swizzled K staging: [P, kt, c*128+n]
                    kst_t = kstpool.tile([P, 4, DC * P], F8, name="kst_t")
                    kt_ps = [pspool.tile([P, R], F32, name=f"ktps{m}", tag="ps")
                             for m in range(DC)]
                    for j in range(4):
                        for m in range(DC):
                            nc.tensor.matmul(
                                kt_ps[m], wk8_t[j][:, :, m * P:(m + 1) * P],
                                xt8_t[j], start=(j == 0), stop=(j == 3),
                                perf_mode=DR)
                    for m in range(DC):
                        nc.vector.tensor_scalar_add(
                            kst_t[:, :, m * P:(m + 1) * P],
                            kt_ps[m].rearrange("p (s n) -> p s n", s=4),
                            bk_t[:, m:m + 1])
                    for h in range(2):
                        nc.scalar.dma_start(
                            kst_d[h][:, :],
                            kst_t[:, 2 * h:2 * h + 2, :]
                            .rearrange("p s n -> p (s n)"))
                    for h in range(2):
                        nc.gpsimd.collective_compute(
                            "AllGather", mybir.AluOpType.bypass,
                            replica_groups=RG,
                            ins=[kst_d[h].ap().opt()],
                            outs=[ag_k[h].ap().opt()])

                    # --- V_c = X_c @ Wv.T + bv  (fp8 DoubleRow) ---
                    v_ps = [pspool.tile([P, R], F32, name=f"vps{o}", tag="ps")
                            for o in range(8)]
                    for j in range(4):
                        for o in range(8):
                            rt, db = o // 2, o % 2
                            nc.tensor.matmul(
                                v_ps[o], xt8_t[j][:, :, rt * P:(rt + 1) * P],
                                wv8_t[j][:, :, db * 512:(db + 1) * 512],
                                start=(j == 0), stop=(j == 3), perf_mode=DR)
                    st_v = []
                    for o in range(8):
                        rt, db = o // 2, o % 2
                        st = stpool.tile([P, 512], F8, name="st_v", tag="st")
                        nc.vector.tensor_add(
                            st, v_ps[o], bvb_t[:, db * 512:(db + 1) * 512])
                        st_v.append(st)

                    # --- Q_c.T = Wq @ X_c.T + bq  (fp16) ---
                    qt_t = [qpool.tile([P, R], F16, name=f"qt{m}")
                            for m in range(DC)]
                    qt8_t = qpool.tile([P, DC * R], F8, name="qt8")
                    q_ps = [pspool.tile([P, R], F32, name=f"q_ps{m}", tag="ps")
                            for m in range(DC)]
                    for k in range(DC):
                        for m in range(DC):
                            nc.tensor.matmul(
                                q_ps[m], wq_t[k][:, m * P:(m + 1) * P],
                                xt_t[k],
                                start=(k == 0), stop=(k == DC - 1))
                    for m in range(DC):
                        nc.vector.tensor_scalar_add(qt_t[m], q_ps[m],
                                                    bq_t[:, m:m + 1])
                        nc.vector.tensor_scalar_add(
                            qt8_t[:, m * R:(m + 1) * R], q_ps[m],
                            bq_t[:, m:m + 1])

                # ---- Q-half fillers while the allgathers fly ----
                x1q_t = [qfpool.tile([P, R], F32, name=f"x1q{m}") for m in range(DC)]
                hqa_t = [qfpool.tile([P, R], F32, name=f"hqa{m}") for m in range(DC)]
                hqb_t = [qfpool.tile([P, R], F32, name=f"hqb{m}") for m in range(DC)]
                with tc.tile_pool(name="fwpool", bufs=1) as fwpool:
                    wlq_t = [fwpool.tile([P, D], F16, name=f"wlq_{k}")
                             for k in range(DC)]
                    waqa_t = [fwpool.tile([P, D], F16, name=f"waqa_{k}")
                              for k in range(DC)]
                    waqb_t = [qfpool.tile([P, D], F16, name=f"waqb_{k}")
                              for k in range(DC)]
                    for k in range(DC):
                        nc.sync.dma_start(
                            wlq_t[k], wlt[(DC + k) * P:(DC + k + 1) * P, :])
                        nc.sync.dma_start(
                            waqa_t[k], wat[(DC + k) * P:(DC + k + 1) * P, 0:D])
                        nc.scalar.dma_start(
                            waqb_t[k], wat[(DC + k) * P:(DC + k + 1) * P, D:TD])
                    # finals weights: on the sync-queue tail, which is idle
                    # after the filler weights and carries no klp traffic
                    wlv_b = vwpool.tile([P, DC * D], F16, name="wlv_b")
                    wava_b = vwpool.tile([P, DC * D], F16, name="wava_b")
                    nc.sync.dma_start(
                        wlv_b.rearrange("p (k c) -> p k c", k=DC),
                        wlt[0:D, :].rearrange("(k p) c -> p k c", p=P))
                    nc.sync.dma_start(
                        wava_b.rearrange("p (k c) -> p k c", k=DC),
                        wat[0:D, 0:D].rearrange("(k p) c -> p k c", p=P))

                    def filler(wts, dst, bias, bias_col, tag):
                        ps = [pspool.tile([P, R], F32, name=f"{tag}{m}",
                                          tag="ps") for m in range(DC)]
                        for k in range(DC):
                            for m in range(DC):
                                nc.tensor.matmul(
                                    ps[m], wts[k][:, m * P:(m + 1) * P],
                                    qt_t[k], start=(k == 0),
                                    stop=(k == DC - 1))
                        for m in range(DC):
                            nc.vector.tensor_scalar_add(
                                dst[m], ps[m],
                                bias[:, bias_col + m:bias_col + m + 1])

                    filler(wlq_t, x1q_t, bl_t, 0, "x1qps")
                    filler(waqa_t, hqa_t, ba_t, 0, "hqaps")
                    hqb_ps7 = [pspool.tile([P, R], F32, name=f"hqbps{m}",
                               tag="ps") for m in range(7)]
                    for k in range(DC):
                        for m in range(7):
                            nc.tensor.matmul(
                                hqb_ps7[m], waqb_t[k][:, m * P:(m + 1) * P],
                                qt_t[k], start=(k == 0), stop=(k == DC - 1))
                    for m in range(7):
                        nc.vector.tensor_scalar_add(
                            hqb_t[m], hqb_ps7[m], ba_t[:, DC + m:DC + m + 1])

                # ============ scoresT + exp + sums (fp8 DR) ============
                # pair pi: kp = pi // 8 (kt-half), rr = pi % 8 (source rank);
                # covers global keys rr*512 + kp*256 + s*128 + p.
                with tc.tile_pool(name="epool", bufs=1) as epool:
                    exp_p = [epool.tile([P, 2 * R], F8, name=f"exp{pi}")
                             for pi in range(NPAIR)]
                    sums_ps = pspool.tile([1, R], F32, name="sums_ps", tag="ps")

                    def sums_mm(pj):
                        nc.tensor.matmul(
                            sums_ps, ones2_t[:, :, 0:1],
                            exp_p[pj].rearrange("p (s n) -> p s n", s=2),
                            start=(pj == 0), stop=(pj == NPAIR - 1),
                            perf_mode=DR, skip_group_check=True)

                    with tc.tile_pool(name="kpool", bufs=1) as kpool:
                        klp_t = []
                        for pi in range(NPAIR):
                            kp, rr = pi // 8, pi % 8
                            klp = kpool.tile([P, 2 * DC * P], F8,
                                             name=f"klp{pi}")
                            # kp0 half on the always-flowing scalar queue;
                            # kp1 half (needed ~20us later) on sync, which
                            # is free once the fillers' bookkeeping drains
                            eng = nc.scalar if kp == 0 else nc.sync
                            eng.dma_start(
                                klp, ag_k[kp][rr * P:(rr + 1) * P, :])
                            klp_t.append(klp)
                        for o in range(8):
                            rt, db = o // 2, o % 2
                            nc.scalar.dma_start(
                                vc_d[rt * P:(rt + 1) * P,
                                     db * 512:(db + 1) * 512], st_v[o])
                        nc.gpsimd.collective_compute(
                            "AllGather", mybir.AluOpType.bypass,
                            replica_groups=RG,
                            ins=[vc_d.ap().opt()], outs=[ag_v.ap().opt()])
                        for pi in range(NPAIR):
                            for s in range(2):
                                off = s * DC * P
                                sc_ps = pspool.tile([P, R], F32, name="sc_ps",
                                                    tag="ps")
                                for cp in range(4):
                                    nc.tensor.matmul(
                                        sc_ps,
                                        klp_t[pi][:, off + 2 * cp * P:
                                                  off + (2 * cp + 2) * P]
                                        .rearrange("p (c n) -> p c n", c=2),
                                        qt8_t[:, 2 * cp * R:(2 * cp + 2) * R]
                                        .rearrange("p (c n) -> p c n", c=2),
                                        start=(cp == 0), stop=(cp == 3),
                                        perf_mode=DR)
                                nc.scalar.activation(
                                    exp_p[pi][:, s * R:(s + 1) * R], sc_ps,
                                    mybir.ActivationFunctionType.Exp,
                                    bias=0.0, scale=1.0 / 32.0)
                                if s == 0 and pi > 0:
                                    sums_mm(pi - 1)
                        sums_mm(NPAIR - 1)

                    # deferred Q-half tail group: fills the sums-tail
                    # window before the broadcast matmul
                    for m in range(7, DC):
                        ps = pspool.tile([P, R], F32, name="hqb_ps", tag="ps")
                        for k in range(DC):
                            nc.tensor.matmul(
                                ps, waqb_t[k][:, m * P:(m + 1) * P], qt_t[k],
                                start=(k == 0), stop=(k == DC - 1))
                        nc.vector.tensor_scalar_add(
                            hqb_t[m], ps, ba_t[:, DC + m:DC + m + 1])

                    # ============ V_T = (alpha @ V).T  (fp8 DR) ============
                    # m-outer with all V tiles resident: each vt group closes
                    # early so its drain overlaps the next group's matmuls.
                    with tc.tile_pool(name="vlpool", bufs=1) as vlpool:
                        vl2_t = []
                        for pi in range(NPAIR):
                            kp, rr = pi // 8, pi % 8
                            vl2 = vlpool.tile([P, 2, D], F8, name=f"vl2_{pi}")
                            base = rr * R + kp * 256
                            eng = nc.sync if pi % 2 == 0 else nc.scalar
                            eng.dma_start(
                                vl2,
                                ag_v[base:base + 256, :]
                                .rearrange("(s p) d -> p s d", p=P))
                            vl2_t.append(vl2)
                        vt_t = [vtpool.tile([P, R], F16, name=f"vt{m}")
                                for m in range(DC)]
                        bc_t = cpool.tile([P, R], F32, name="bc_t")
                        for m in range(DC):
                            vt_ps = pspool.tile([P, R], F32, name="vt_ps",
                                                tag="ps")
                            for pi in range(NPAIR):
                                nc.tensor.matmul(
                                    vt_ps, vl2_t[pi][:, :, m * P:(m + 1) * P],
                                    exp_p[pi].rearrange("p (s n) -> p s n", s=2),
                                    start=(pi == 0), stop=(pi == NPAIR - 1),
                                    perf_mode=DR)
                            if m == 0:
                                # recip + partition-broadcast of 1/sums: its
                                # wait on the last exp overlaps the m=0 MMs
                                recip_t = cpool.tile([1, R], F16,
                                                     name="recip_t")
                                with nc.allow_low_precision(
                                        reason="1/sums fp16 is plenty"):
                                    nc.vector.reciprocal(recip_t, sums_ps)
                                bc_ps = pspool.tile([P, R], F32, name="bc_ps",
                                                    tag="ps")
                                nc.tensor.matmul(bc_ps, ones_row, recip_t,
                                                 start=True, stop=True)
                                nc.vector.tensor_copy(bc_t, bc_ps)
                            nc.vector.tensor_mul(vt_t[m], vt_ps, bc_t)

                # ============ x1 (V-half), h (V-half), GLU ============
                with tc.tile_pool(name="fpool", bufs=1) as fpool, \
                     tc.tile_pool(name="vwbpool", bufs=1) as vwbpool, \
                     tc.tile_pool(name="opool", bufs=2) as opool:
                    wavb_b = vwbpool.tile([P, DC * D], F16, name="wavb_b")
                    nc.scalar.dma_start(
                        wavb_b.rearrange("p (k c) -> p k c", k=DC),
                        wat[0:D, D:TD].rearrange("(k p) c -> p k c", p=P))

                    x1_t = [fpool.tile([P, R], F32, name=f"x1{m}") for m in range(DC)]
                    for m in range(DC):
                        ps = pspool.tile([P, R], F32, name="x1_ps", tag="ps")
                        for k in range(DC):
                            nc.tensor.matmul(
                                ps, wlv_b[:, k * D + m * P:k * D + (m + 1) * P],
                                vt_t[k], start=(k == 0), stop=(k == DC - 1))
                        nc.vector.tensor_add(x1_t[m], ps, x1q_t[m])

                    a_t = [fpool.tile([P, R], F32, name=f"a{m}") for m in range(DC)]
                    for m in range(DC):
                        ps = pspool.tile([P, R], F32, name="hva_ps", tag="ps")
                        for k in range(DC):
                            nc.tensor.matmul(
                                ps, wava_b[:, k * D + m * P:k * D + (m + 1) * P],
                                vt_t[k], start=(k == 0), stop=(k == DC - 1))
                        nc.vector.tensor_add(a_t[m], ps, hqa_t[m])

                    # b half m-outer so GLU/output tail overlaps remaining MMs
                    for m in range(DC):
                        ps = pspool.tile([P, R], F32, name="hvb_ps", tag="ps")
                        for k in range(DC):
                            nc.tensor.matmul(
                                ps, wavb_b[:, k * D + m * P:k * D + (m + 1) * P],
                                vt_t[k], start=(k == 0), stop=(k == DC - 1))
                        b_t = opool.tile([P, R], F32, name="b_t", tag="bt")
                        nc.vector.tensor_add(b_t, ps, hqb_t[m])
                        sig = opool.tile([P, R], F32, name="sig", tag="sig")
                        nc.scalar.activation(
                            sig, b_t, mybir.ActivationFunctionType.Sigmoid,
                            bias=0.0, scale=1.0)
                        nc.vector.tensor_mul(a_t[m], a_t[m], sig)
                        nc.vector.tensor_mul(a_t[m], x1_t[m], a_t[m])
                        nc.scalar.dma_start(out[m * P:(m + 1) * P, :], a_t[m])

    nc.compile()
    return nc


_NC = None


def _get_nc():
    global _NC
    if _NC is None:
        _NC = build_nc()
    return _NC


def make_in_maps(input_features, Wq, bq, Wk, bk, Wv, bv, Wl, bl, Wa, ba):
    f = np.ascontiguousarray
    FP8 = ml_dtypes.float8_e4m3
    x = np.asarray(input_features, dtype=np.float32)
    xt_full = f(x.T.astype(np.float16))                  # [D, N]
    xt8_full = f(x.T.astype(FP8))
    wqt = f(np.asarray(Wq, np.float32).T.astype(np.float16))
    wk8 = f(np.asarray(Wk, np.float32).T.astype(FP8))
    wv8 = f(np.asarray(Wv, np.float32).T.astype(FP8))
    wlt = f(np.asarray(Wl, np.float32).T.astype(np.float16))   # [2D, D]
    wat = f(np.asarray(Wa, np.float32).T.astype(np.float16))   # [2D, 2D]
    bq_r = f(np.asarray(bq, np.float32).reshape(DC, P).T)      # [P, DC]
    bk_r = f(np.asarray(bk, np.float32).reshape(DC, P).T)
    bl_r = f(np.asarray(bl, np.float32).reshape(DC, P).T)
    ba_r = f(np.asarray(ba, np.float32).reshape(TDC, P).T)     # [P, TDC]
    bvb = f(np.broadcast_to(np.asarray(bv, np.float32), (P, D)))
    in_maps = []
    for c in range(NCORES):
        in_maps.append({
            "xt": f(xt_full[:, c * R:(c + 1) * R]),
            "xt8": f(xt8_full[:, c * R:(c + 1) * R]),
            "wqt": wqt, "wk8": wk8, "wv8": wv8, "wlt": wlt, "wat": wat,
            "bq": bq_r, "bk": bk_r, "bvb": bvb, "bl": bl_r, "ba": ba_r,
        })
    return in_maps


def run(in_maps, trace=False):
    nc = _get_nc()
    return bass_utils.run_bass_kernel_spmd(
        nc, in_maps, core_ids=list(range(NCORES)), trace=trace)


def kernel(input_features, Wq, bq, Wk, bk, Wv, bv, Wl, bl, Wa, ba):
    in_maps = make_in_maps(input_features, Wq, bq, Wk, bk, Wv, bv, Wl, bl, Wa, ba)
    res = run(in_maps)
    out = np.empty((N, D), dtype=np.float32)
    for c in range(NCORES):
        out[c * R:(c + 1) * R, :] = res.results[c]["out"].T
    return out
